# revision 4
# baseline (speedup 1.0000x reference)
"""LSTM chatbot model (embed -> LSTM -> vocab projection) on 8 trn2 cores.

Sharding: embedding + LSTM replicated on all cores (the recurrence is
latency-bound, not FLOP-bound, so data-parallelism does not help it);
the large logits projection is tensor-parallel over vocab (4000 rows of
W_fc per core). Each core writes its own [4096, 4000] logits shard and
the host concatenates. No collectives.

All GEMMs run in float32r (TF32-like, ~1.6e-4 rel err, full PE speed).
"""

from contextlib import ExitStack

import numpy as np

import concourse.bass as bass
import concourse.mybir as mybir
import concourse.tile as tile
from concourse import bacc, bass_utils
from concourse.masks import make_identity

S, B, H, V = 128, 32, 512, 32000
G = 4 * H          # 2048 gates
SB = S * B         # 4096 tokens
NCORES = 8
VS = V // NCORES   # 4000 vocab rows per core

F32 = mybir.dt.float32
F32R = mybir.dt.float32r
I32 = mybir.dt.int32
AF = mybir.ActivationFunctionType

_CACHE = {}


def _emit(nc, tc, xi, emb, wih, whh, biasg, wfc, bfc, logits):
    ctx = ExitStack()
    with ctx:
        # ---------------- persistent tiles ----------------
        const = ctx.enter_context(tc.tile_pool(name="const", bufs=1))
        id128 = const.tile([128, 128], F32)
        make_identity(nc, id128[:])
        id32f = const.tile([32, 32], F32)
        make_identity(nc, id32f[:])
        id32r = const.tile([32, 32], F32R)
        nc.vector.tensor_copy(id32r[:], id32f[:])

        idx_sb = const.tile([128, 32], I32)
        for m in range(32):
            nc.sync.dma_start(idx_sb[:, m : m + 1], xi[128 * m : 128 * (m + 1), :])

        whh_pool = ctx.enter_context(tc.tile_pool(name="whh", bufs=1))
        whh_sb = [whh_pool.tile([128, G], F32R, name=f"whh{k}") for k in range(4)]
        for k in range(4):
            nc.sync.dma_start(whh_sb[k][:], whh[128 * k : 128 * (k + 1), :])

        state = ctx.enter_context(tc.tile_pool(name="state", bufs=1))
        hsT = state.tile([128, 4, SB], F32R)  # transposed hidden states, 8 MB
        c_sb = state.tile([32, H], F32)       # cell state
        nc.vector.memset(c_sb[:], 0.0)

        dram = ctx.enter_context(tc.tile_pool(name="dram", bufs=1, space="DRAM"))
        xg_dram = dram.tile([SB, G], F32R)    # precomputed input gates

        # ---------------- phase B: gather + x_gates ----------------
        with tc.tile_pool(name="wih", bufs=1) as wih_pool, \
             tc.tile_pool(name="bconst", bufs=1) as bconst, \
             tc.tile_pool(name="bwork", bufs=3) as bwork, \
             tc.tile_pool(name="bpt", bufs=2, space="PSUM") as bpt_pool, \
             tc.tile_pool(name="bpg", bufs=5, space="PSUM") as bpg_pool:
            wih_sb = [wih_pool.tile([128, G], F32R, name=f"wih{k}") for k in range(4)]
            for k in range(4):
                nc.sync.dma_start(wih_sb[k][:], wih[128 * k : 128 * (k + 1), :])
            bias_sb = bconst.tile([128, G], F32)
            nc.sync.dma_start(bias_sb[:], biasg[:])

            for m in range(32):
                ms = slice(128 * m, 128 * (m + 1))
                emb_m = bwork.tile([128, H], F32, tag="emb_m")
                nc.gpsimd.indirect_dma_start(
                    out=emb_m[:],
                    out_offset=None,
                    in_=emb[:],
                    in_offset=bass.IndirectOffsetOnAxis(
                        ap=idx_sb[:, m : m + 1], axis=0
                    ),
                )
                pt = bpt_pool.tile([128, H], F32)
                for u in range(4):
                    nc.tensor.transpose(
                        pt[:, 128 * u : 128 * (u + 1)],
                        emb_m[:, 128 * u : 128 * (u + 1)],
                        id128[:],
                    )
                embT = bwork.tile([128, H], F32R, tag="embT")
                nc.vector.tensor_copy(embT[:], pt[:])
                pg = [bpg_pool.tile([128, 512], F32, name="pg", tag="pg") for _ in range(4)]
                for k in range(4):
                    lhs = embT[:, 128 * k : 128 * (k + 1)]
                    for n in range(4):
                        nc.tensor.matmul(
                            pg[n][:],
                            lhs,
                            wih_sb[k][:, 512 * n : 512 * (n + 1)],
                            start=(k == 0),
                            stop=(k == 3),
                        )
                for n in range(4):
                    xo = bwork.tile([128, 512], F32R, tag="xo")
                    nc.vector.tensor_add(
                        xo[:], pg[n][:], bias_sb[:, 512 * n : 512 * (n + 1)]
                    )
                    nc.sync.dma_start(
                        xg_dram[ms, 512 * n : 512 * (n + 1)], xo[:]
                    )

        # ---------------- phase C: LSTM recurrence ----------------
        with tc.tile_pool(name="cwork", bufs=3) as cwork, \
             tc.tile_pool(name="cpg", bufs=1, space="PSUM") as cpg_pool, \
             tc.tile_pool(name="cph", bufs=2, space="PSUM") as cph_pool:
            for s in range(S):
                xg_s = cwork.tile([32, G], F32R, tag="xg_s")
                nc.sync.dma_start(xg_s[:], xg_dram[32 * s : 32 * (s + 1), :])
                pgt = cpg_pool.tile([32, G], F32)
                for n in range(4):
                    nc.tensor.matmul(
                        pgt[:, 512 * n : 512 * (n + 1)],
                        id32r[:],
                        xg_s[:, 512 * n : 512 * (n + 1)],
                        start=True,
                        stop=(s == 0),
                    )
                if s > 0:
                    hT_prev = hsT[:, :, 32 * (s - 1) : 32 * s]
                    for k in range(4):
                        lhs = hT_prev[:, k, :]
                        for n in range(4):
                            nc.tensor.matmul(
                                pgt[:, 512 * n : 512 * (n + 1)],
                                lhs,
                                whh_sb[k][:, 512 * n : 512 * (n + 1)],
                                start=False,
                                stop=(k == 3),
                            )
                i_sb = cwork.tile([32, H], F32, tag="i_sb")
                f_sb = cwork.tile([32, H], F32, tag="f_sb")
                g_sb = cwork.tile([32, H], F32, tag="g_sb")
                o_sb = cwork.tile([32, H], F32, tag="o_sb")
                nc.scalar.activation(i_sb[:], pgt[:, 0:512], AF.Sigmoid)
                nc.scalar.activation(f_sb[:], pgt[:, 512:1024], AF.Sigmoid)
                nc.scalar.activation(g_sb[:], pgt[:, 1024:1536], AF.Tanh)
                nc.scalar.activation(o_sb[:], pgt[:, 1536:2048], AF.Sigmoid)
                ig = cwork.tile([32, H], F32, tag="ig")
                fc = cwork.tile([32, H], F32, tag="fc")
                nc.vector.tensor_mul(ig[:], i_sb[:], g_sb[:])
                nc.vector.tensor_mul(fc[:], f_sb[:], c_sb[:])
                nc.vector.tensor_add(c_sb[:], ig[:], fc[:])
                th = cwork.tile([32, H], F32, tag="th")
                nc.scalar.activation(th[:], c_sb[:], AF.Tanh)
                h_sb = cwork.tile([32, H], F32, tag="h_sb")
                nc.vector.tensor_mul(h_sb[:], o_sb[:], th[:])
                ph = cph_pool.tile([128, 4, 32], F32)
                for u in range(4):
                    nc.tensor.transpose(
                        ph[:, u, :], h_sb[:, 128 * u : 128 * (u + 1)], id32f[:]
                    )
                nc.vector.tensor_copy(hsT[:, :, 32 * s : 32 * (s + 1)], ph[:])

        # ---------------- phase D: logits ----------------
        with tc.tile_pool(name="wfc", bufs=1) as wfc_pool, \
             tc.tile_pool(name="dwork", bufs=4) as dwork, \
             tc.tile_pool(name="dpl", bufs=4, space="PSUM") as dpl_pool:
            wfc_sb = [wfc_pool.tile([128, VS], F32R, name=f"wfc{k}") for k in range(4)]
            for k in range(4):
                nc.sync.dma_start(wfc_sb[k][:], wfc[128 * k : 128 * (k + 1), :])
            bfc_sb = wfc_pool.tile([128, VS], F32)
            nc.sync.dma_start(bfc_sb[:], bfc[:])

            for m in range(32):
                ms = slice(128 * m, 128 * (m + 1))
                for n in range(8):
                    ns = slice(500 * n, 500 * (n + 1))
                    pl = dpl_pool.tile([128, 500], F32)
                    for k in range(4):
                        nc.tensor.matmul(
                            pl[:],
                            hsT[:, k, ms],
                            wfc_sb[k][:, ns],
                            start=(k == 0),
                            stop=(k == 3),
                        )
                    ol = dwork.tile([128, 500], F32, tag="ol")
                    nc.vector.tensor_add(ol[:], pl[:], bfc_sb[:, ns])
                    nc.sync.dma_start(logits[ms, ns], ol[:])


def _build():
    nc = bacc.Bacc(
        "TRN2", target_bir_lowering=False, debug=False, num_devices=NCORES
    )
    xi = nc.dram_tensor("xi", [SB, 1], I32, kind="ExternalInput").ap()
    emb = nc.dram_tensor("emb", [V, H], F32, kind="ExternalInput").ap()
    wih = nc.dram_tensor("wih", [H, G], F32R, kind="ExternalInput").ap()
    whh = nc.dram_tensor("whh", [H, G], F32R, kind="ExternalInput").ap()
    biasg = nc.dram_tensor("biasg", [128, G], F32, kind="ExternalInput").ap()
    wfc = nc.dram_tensor("wfc", [H, VS], F32R, kind="ExternalInput").ap()
    bfc = nc.dram_tensor("bfc", [128, VS], F32, kind="ExternalInput").ap()
    logits = nc.dram_tensor("logits", [SB, VS], F32, kind="ExternalOutput").ap()
    with tile.TileContext(nc) as tc:
        _emit(nc, tc, xi, emb, wih, whh, biasg, wfc, bfc, logits)
    nc.compile()
    return nc


def _get_nc():
    if "nc" not in _CACHE:
        _CACHE["nc"] = _build()
    return _CACHE["nc"]


def kernel(x, emb_table, W_ih, W_hh, b_ih, b_hh, W_fc, b_fc):
    x = np.asarray(x)
    emb_table = np.ascontiguousarray(np.asarray(emb_table, dtype=np.float32))
    W_ih = np.asarray(W_ih, dtype=np.float32)
    W_hh = np.asarray(W_hh, dtype=np.float32)
    b_ih = np.asarray(b_ih, dtype=np.float32)
    b_hh = np.asarray(b_hh, dtype=np.float32)
    W_fc = np.asarray(W_fc, dtype=np.float32)
    b_fc = np.asarray(b_fc, dtype=np.float32)

    xi = x.reshape(SB, 1).astype(np.int32)
    wih_t = np.ascontiguousarray(W_ih.T)            # [512, 2048]
    whh_t = np.ascontiguousarray(W_hh.T)            # [512, 2048]
    biasg = np.tile((b_ih + b_hh)[None, :], (128, 1))

    nc = _get_nc()
    in_maps = []
    for c in range(NCORES):
        wfc_t = np.ascontiguousarray(W_fc[VS * c : VS * (c + 1)].T)  # [512, 4000]
        bfc_b = np.tile(b_fc[VS * c : VS * (c + 1)][None, :], (128, 1))
        in_maps.append(
            {
                "xi": xi,
                "emb": emb_table,
                "wih": wih_t,
                "whh": whh_t,
                "biasg": biasg,
                "wfc": wfc_t,
                "bfc": bfc_b,
            }
        )
    res = bass_utils.run_bass_kernel_spmd(
        nc, in_maps, core_ids=list(range(NCORES))
    )
    shards = [
        res.results[c]["logits"].reshape(S, B, VS) for c in range(NCORES)
    ]
    return np.concatenate(shards, axis=2)


# revision 6
# speedup vs baseline: 249.9802x; 249.9802x over previous
"""LSTM chatbot model (embed -> LSTM -> vocab projection) on 8 trn2 cores.

Sharding: embedding + LSTM replicated on all cores (the recurrence is
latency-bound, not FLOP-bound, so data-parallelism does not help it);
the large logits projection is tensor-parallel over vocab (4000 rows of
W_fc per core). Each core writes its own [4096, 4000] logits shard and
the host concatenates. No collectives.

All GEMMs run in float32r (TF32-like, ~1.6e-4 rel err, full PE speed).
"""

from contextlib import ExitStack

import numpy as np

import concourse.bass as bass
import concourse.mybir as mybir
import concourse.tile as tile
from concourse import bacc, bass_utils
from concourse.masks import make_identity

S, B, H, V = 128, 32, 512, 32000
G = 4 * H          # 2048 gates
SB = S * B         # 4096 tokens
NCORES = 8
VS = V // NCORES   # 4000 vocab rows per core

F32 = mybir.dt.float32
F32R = mybir.dt.float32r
I32 = mybir.dt.int32
AF = mybir.ActivationFunctionType

_CACHE = {}


def _emit(nc, tc, xi, emb, wih, whh, biasg, wfc, bfc, logits):
    ctx = ExitStack()
    with ctx:
        # ---------------- persistent tiles ----------------
        const = ctx.enter_context(tc.tile_pool(name="const", bufs=1))
        id128 = const.tile([128, 128], F32)
        make_identity(nc, id128[:])
        id32f = const.tile([32, 32], F32)
        make_identity(nc, id32f[:])
        id32r = const.tile([32, 32], F32R)
        nc.vector.tensor_copy(id32r[:], id32f[:])

        idx_sb = const.tile([128, 32], I32)
        for m in range(32):
            nc.sync.dma_start(idx_sb[:, m : m + 1], xi[128 * m : 128 * (m + 1), :])

        whh_pool = ctx.enter_context(tc.tile_pool(name="whh", bufs=1))
        whh_sb = [whh_pool.tile([128, G], F32R, name=f"whh{k}") for k in range(4)]
        for k in range(4):
            nc.sync.dma_start(whh_sb[k][:], whh[128 * k : 128 * (k + 1), :])

        state = ctx.enter_context(tc.tile_pool(name="state", bufs=1))
        hsT = state.tile([128, 4, SB], F32R)  # transposed hidden states, 8 MB
        c_sb = state.tile([32, H], F32)       # cell state
        nc.vector.memset(c_sb[:], 0.0)

        dram = ctx.enter_context(tc.tile_pool(name="dram", bufs=1, space="DRAM"))
        xg_dram = dram.tile([SB, G], F32R)    # precomputed input gates

        # ---------------- phase B: gather + x_gates ----------------
        with tc.tile_pool(name="wih", bufs=1) as wih_pool, \
             tc.tile_pool(name="bconst", bufs=1) as bconst, \
             tc.tile_pool(name="bwork", bufs=3) as bwork, \
             tc.tile_pool(name="bpt", bufs=2, space="PSUM") as bpt_pool, \
             tc.tile_pool(name="bpg", bufs=5, space="PSUM") as bpg_pool:
            wih_sb = [wih_pool.tile([128, G], F32R, name=f"wih{k}") for k in range(4)]
            for k in range(4):
                nc.sync.dma_start(wih_sb[k][:], wih[128 * k : 128 * (k + 1), :])
            bias_sb = bconst.tile([128, G], F32)
            nc.sync.dma_start(bias_sb[:], biasg[:])

            for m in range(32):
                ms = slice(128 * m, 128 * (m + 1))
                emb_m = bwork.tile([128, H], F32, tag="emb_m")
                nc.gpsimd.indirect_dma_start(
                    out=emb_m[:],
                    out_offset=None,
                    in_=emb[:],
                    in_offset=bass.IndirectOffsetOnAxis(
                        ap=idx_sb[:, m : m + 1], axis=0
                    ),
                )
                pt = bpt_pool.tile([128, H], F32)
                for u in range(4):
                    nc.tensor.transpose(
                        pt[:, 128 * u : 128 * (u + 1)],
                        emb_m[:, 128 * u : 128 * (u + 1)],
                        id128[:],
                    )
                embT = bwork.tile([128, H], F32R, tag="embT")
                nc.vector.tensor_copy(embT[:], pt[:])
                pg = [bpg_pool.tile([128, 512], F32, name="pg", tag="pg") for _ in range(4)]
                for k in range(4):
                    lhs = embT[:, 128 * k : 128 * (k + 1)]
                    for n in range(4):
                        nc.tensor.matmul(
                            pg[n][:],
                            lhs,
                            wih_sb[k][:, 512 * n : 512 * (n + 1)],
                            start=(k == 0),
                            stop=(k == 3),
                        )
                for n in range(4):
                    xo = bwork.tile([128, 512], F32R, tag="xo")
                    nc.vector.tensor_add(
                        xo[:], pg[n][:], bias_sb[:, 512 * n : 512 * (n + 1)]
                    )
                    nc.sync.dma_start(
                        xg_dram[ms, 512 * n : 512 * (n + 1)], xo[:]
                    )

        # ---------------- phase C: LSTM recurrence ----------------
        with tc.tile_pool(name="cwork", bufs=3) as cwork, \
             tc.tile_pool(name="cpg", bufs=1, space="PSUM") as cpg_pool, \
             tc.tile_pool(name="cph", bufs=2, space="PSUM") as cph_pool:
            for s in range(S):
                xg_s = cwork.tile([32, G], F32R, tag="xg_s")
                nc.sync.dma_start(xg_s[:], xg_dram[32 * s : 32 * (s + 1), :])
                pgt = cpg_pool.tile([32, G], F32)
                for n in range(4):
                    nc.tensor.matmul(
                        pgt[:, 512 * n : 512 * (n + 1)],
                        id32r[:],
                        xg_s[:, 512 * n : 512 * (n + 1)],
                        start=True,
                        stop=(s == 0),
                    )
                if s > 0:
                    hT_prev = hsT[:, :, 32 * (s - 1) : 32 * s]
                    for k in range(4):
                        lhs = hT_prev[:, k, :]
                        for n in range(4):
                            nc.tensor.matmul(
                                pgt[:, 512 * n : 512 * (n + 1)],
                                lhs,
                                whh_sb[k][:, 512 * n : 512 * (n + 1)],
                                start=False,
                                stop=(k == 3),
                            )
                i_sb = cwork.tile([32, H], F32, tag="i_sb")
                f_sb = cwork.tile([32, H], F32, tag="f_sb")
                g_sb = cwork.tile([32, H], F32, tag="g_sb")
                o_sb = cwork.tile([32, H], F32, tag="o_sb")
                nc.scalar.activation(i_sb[:], pgt[:, 0:512], AF.Sigmoid)
                nc.scalar.activation(f_sb[:], pgt[:, 512:1024], AF.Sigmoid)
                nc.scalar.activation(g_sb[:], pgt[:, 1024:1536], AF.Tanh)
                nc.scalar.activation(o_sb[:], pgt[:, 1536:2048], AF.Sigmoid)
                ig = cwork.tile([32, H], F32, tag="ig")
                fc = cwork.tile([32, H], F32, tag="fc")
                nc.vector.tensor_mul(ig[:], i_sb[:], g_sb[:])
                nc.vector.tensor_mul(fc[:], f_sb[:], c_sb[:])
                nc.vector.tensor_add(c_sb[:], ig[:], fc[:])
                th = cwork.tile([32, H], F32, tag="th")
                nc.scalar.activation(th[:], c_sb[:], AF.Tanh)
                h_sb = cwork.tile([32, H], F32, tag="h_sb")
                nc.vector.tensor_mul(h_sb[:], o_sb[:], th[:])
                ph = cph_pool.tile([128, 4, 32], F32)
                for u in range(4):
                    nc.tensor.transpose(
                        ph[:, u, :], h_sb[:, 128 * u : 128 * (u + 1)], id32f[:]
                    )
                nc.vector.tensor_copy(hsT[:, :, 32 * s : 32 * (s + 1)], ph[:])

        # ---------------- phase D: logits ----------------
        with tc.tile_pool(name="wfc", bufs=1) as wfc_pool, \
             tc.tile_pool(name="dwork", bufs=4) as dwork, \
             tc.tile_pool(name="dpl", bufs=4, space="PSUM") as dpl_pool:
            wfc_sb = [wfc_pool.tile([128, VS], F32R, name=f"wfc{k}") for k in range(4)]
            for k in range(4):
                nc.sync.dma_start(wfc_sb[k][:], wfc[128 * k : 128 * (k + 1), :])
            bfc_sb = wfc_pool.tile([128, VS], F32)
            nc.sync.dma_start(bfc_sb[:], bfc[:])

            for m in range(32):
                ms = slice(128 * m, 128 * (m + 1))
                for n in range(8):
                    ns = slice(500 * n, 500 * (n + 1))
                    pl = dpl_pool.tile([128, 500], F32)
                    for k in range(4):
                        nc.tensor.matmul(
                            pl[:],
                            hsT[:, k, ms],
                            wfc_sb[k][:, ns],
                            start=(k == 0),
                            stop=(k == 3),
                        )
                    ol = dwork.tile([128, 500], F32, tag="ol")
                    nc.vector.tensor_add(ol[:], pl[:], bfc_sb[:, ns])
                    nc.sync.dma_start(logits[ms, ns], ol[:])


def _build():
    nc = bacc.Bacc(
        "TRN2", target_bir_lowering=False, debug=False, num_devices=NCORES
    )
    xi = nc.dram_tensor("xi", [SB, 1], I32, kind="ExternalInput").ap()
    emb = nc.dram_tensor("emb", [SB, H], F32, kind="ExternalInput").ap()
    wih = nc.dram_tensor("wih", [H, G], F32R, kind="ExternalInput").ap()
    whh = nc.dram_tensor("whh", [H, G], F32R, kind="ExternalInput").ap()
    biasg = nc.dram_tensor("biasg", [128, G], F32, kind="ExternalInput").ap()
    wfc = nc.dram_tensor("wfc", [H, VS], F32R, kind="ExternalInput").ap()
    bfc = nc.dram_tensor("bfc", [128, VS], F32, kind="ExternalInput").ap()
    logits = nc.dram_tensor("logits", [SB, VS], F32, kind="ExternalOutput").ap()
    with tile.TileContext(nc) as tc:
        _emit(nc, tc, xi, emb, wih, whh, biasg, wfc, bfc, logits)
    nc.compile()
    return nc


def _get_nc():
    if "nc" not in _CACHE:
        _CACHE["nc"] = _build()
    return _CACHE["nc"]


def _get_runner():
    """Build the shard_map'd PJRT callable once (mirrors
    bass2jax.run_bass_via_pjrt) so repeat calls skip re-tracing."""
    if "runner" in _CACHE:
        return _CACHE["runner"]
    import jax
    import jax.numpy as jnp
    from jax.sharding import Mesh, PartitionSpec
    from jax.experimental.shard_map import shard_map
    from concourse import bass2jax, mybir as mb

    nc = _get_nc()
    bass2jax.install_neuronx_cc_hook()
    assert nc.dbg_addr is None
    part_name = (
        nc.partition_id_tensor.name if nc.partition_id_tensor else None
    )

    in_names, out_names, out_avals = [], [], []
    for alloc in nc.m.functions[0].allocations:
        if not isinstance(alloc, mb.MemoryLocationSet):
            continue
        name = alloc.memorylocations[0].name
        if alloc.kind == "ExternalInput":
            if name != part_name:
                in_names.append(name)
        elif alloc.kind == "ExternalOutput":
            out_names.append(name)
            out_avals.append(
                jax.core.ShapedArray(
                    tuple(alloc.tensor_shape), mb.dt.np(alloc.dtype)
                )
            )
    n_params = len(in_names)
    n_outs = len(out_avals)
    all_names = in_names + out_names
    if part_name is not None:
        all_names = all_names + [part_name]
    donate = tuple(range(n_params, n_params + n_outs))

    def _body(*args):
        operands = list(args)
        if part_name is not None:
            operands.append(bass2jax.partition_id_tensor())
        outs = bass2jax._bass_exec_p.bind(
            *operands,
            out_avals=tuple(out_avals),
            in_names=tuple(all_names),
            out_names=tuple(out_names),
            lowering_input_output_aliases=(),
            sim_require_finite=True,
            sim_require_nnan=True,
            nc=nc,
        )
        return tuple(outs)

    devices = jax.devices()[:NCORES]
    mesh = Mesh(np.asarray(devices), ("core",))
    in_specs = (PartitionSpec("core"),) * (n_params + n_outs)
    out_specs = (PartitionSpec("core"),) * n_outs
    sharded = jax.jit(
        shard_map(
            _body, mesh=mesh, in_specs=in_specs, out_specs=out_specs,
            check_rep=False,
        ),
        donate_argnums=donate,
        keep_unused=True,
    )
    runner = {
        "jit": sharded,
        "in_names": in_names,
        "out_names": out_names,
        "out_avals": out_avals,
        "jax": jax,
    }
    _CACHE["runner"] = runner
    return runner


def _stage_inputs(in_maps):
    """Concatenate per-core inputs along axis 0 and put on devices."""
    r = _get_runner()
    jax = r["jax"]
    concat = [
        np.concatenate([np.asarray(m[name]) for m in in_maps], axis=0)
        for name in r["in_names"]
    ]
    return [jax.device_put(a) for a in concat]


def _fresh_outs():
    r = _get_runner()
    return [
        np.zeros((NCORES * av.shape[0], *av.shape[1:]), av.dtype)
        for av in r["out_avals"]
    ]


def _execute(ins_dev, outs):
    """One kernel execution. `outs` are donated buffers (consumed);
    returns device output arrays usable as next call's `outs`."""
    r = _get_runner()
    out_arrs = r["jit"](*ins_dev, *outs)
    for a in out_arrs:
        a.block_until_ready()
    return list(out_arrs)


def _make_in_maps(x, emb_table, W_ih, W_hh, b_ih, b_hh, W_fc, b_fc):
    x = np.asarray(x)
    emb_table = np.asarray(emb_table, dtype=np.float32)
    W_ih = np.asarray(W_ih, dtype=np.float32)
    W_hh = np.asarray(W_hh, dtype=np.float32)
    b_ih = np.asarray(b_ih, dtype=np.float32)
    b_hh = np.asarray(b_hh, dtype=np.float32)
    W_fc = np.asarray(W_fc, dtype=np.float32)
    b_fc = np.asarray(b_fc, dtype=np.float32)

    # Dedupe the embedding table: ship only the rows this batch touches
    # (padded to SB rows); the device still gathers per-token rows.
    x_flat = x.reshape(SB).astype(np.int64)
    uniq, inv = np.unique(x_flat, return_inverse=True)
    emb_used = np.zeros((SB, H), np.float32)
    emb_used[: uniq.size] = emb_table[uniq]
    xi = inv.reshape(SB, 1).astype(np.int32)

    wih_t = np.ascontiguousarray(W_ih.T)            # [512, 2048]
    whh_t = np.ascontiguousarray(W_hh.T)            # [512, 2048]
    biasg = np.tile((b_ih + b_hh)[None, :], (128, 1))

    in_maps = []
    for c in range(NCORES):
        wfc_t = np.ascontiguousarray(W_fc[VS * c : VS * (c + 1)].T)
        bfc_b = np.tile(b_fc[VS * c : VS * (c + 1)][None, :], (128, 1))
        in_maps.append(
            {
                "xi": xi,
                "emb": emb_used,
                "wih": wih_t,
                "whh": whh_t,
                "biasg": biasg,
                "wfc": wfc_t,
                "bfc": bfc_b,
            }
        )
    return in_maps


def kernel(x, emb_table, W_ih, W_hh, b_ih, b_hh, W_fc, b_fc):
    in_maps = _make_in_maps(x, emb_table, W_ih, W_hh, b_ih, b_hh, W_fc, b_fc)
    ins_dev = _stage_inputs(in_maps)
    out_arrs = _execute(ins_dev, _fresh_outs())
    r = _get_runner()
    full = np.asarray(out_arrs[r["out_names"].index("logits")])
    shards = full.reshape(NCORES, SB, VS)
    return np.concatenate(
        [shards[c].reshape(S, B, VS) for c in range(NCORES)], axis=2
    )
